# revision 3
# baseline (speedup 1.0000x reference)
"""ArcFace loss on 8 TRN2 NeuronCores (batch-parallel Bass/Tile kernel).

Math: for non-target classes cos(arccos(x)) == x, so logits are just
SCALE*x everywhere except the B target entries, which get
SCALE*(x*cos(m) - sqrt(1-x^2)*sin(m)).  Since cosine < 0.99 strictly,
SCALE*0.99 is an upper bound of every logit, so a constant shift
K = SCALE*0.99 replaces the per-row max (logsumexp is shift-invariant)
and the [B, C] pass is a single streamed exp-accumulate:

    S_all[b]  = sum_c exp(SCALE*x[b,c] - K)           (device, streamed)
    lt[b]     = SCALE*(xt*cos(m) - sqrt(1-xt^2)*sin(m))
    S_true[b] = S_all - exp(SCALE*xt - K) + exp(lt - K)
    loss      = mean_b [ log(S_true) + K - lt ]

Sharding: batch dimension B=2048 -> 256 rows per core (zero-copy host
shards).  Each core streams its [256, 100000] f32 shard (102.4 MB)
through SBUF; ScalarE does exp + free-axis accumulation in a single
ACTIVATE per tile (accum_out), so the pass is purely DMA-bound
(~358 GB/s/core HBM -> ~286 us floor).  The epilogue computes the
margin correction for its 256 rows, reduces to a partial mean, and a
4-byte AllReduce(add) combines the 8 partial means.
"""

import math

import numpy as np

B = 2048
C = 100000
N_CORES = 8
B_PER = B // N_CORES  # 256 rows per core
RB = B_PER // 128  # 2 row-blocks of 128 partitions
CT = 8  # col-tiles per row-block
F = C // CT  # 12500 elements free dim per tile

MARGIN = 0.1
SCALE = 64.0
K_SHIFT = SCALE * 0.99  # upper bound of all logits; constant lse shift

_CACHE = {}


def build_bass(b_per=B_PER, c=C, ct=CT, n_cores=N_CORES, bufs=3):
    """Build + compile the SPMD Bass graph for one core (all cores identical)."""
    import concourse.bacc as bacc
    import concourse.bass as bass
    import concourse.tile as tile
    from concourse import mybir

    f32 = mybir.dt.float32
    AF = mybir.ActivationFunctionType
    rb = b_per // 128
    f = c // ct
    cos_m = float(np.float32(math.cos(MARGIN)))
    sin_m = float(np.float32(math.sin(MARGIN)))

    nc = bacc.Bacc(
        "TRN2",
        target_bir_lowering=False,
        debug=False,
        num_devices=n_cores,
    )
    cos_ext = nc.dram_tensor("cosine", [b_per, c], f32, kind="ExternalInput")
    xt_ext = nc.dram_tensor("xt", [128, rb], f32, kind="ExternalInput")
    out_ext = nc.dram_tensor("out", [1, 1], f32, kind="ExternalOutput")

    with tile.TileContext(nc) as tc:
        with (
            tc.tile_pool(name="stream", bufs=bufs) as stream_pool,
            tc.tile_pool(name="small", bufs=1) as small,
            tc.tile_pool(name="psum", bufs=1, space="PSUM") as psum,
            tc.tile_pool(name="dram", bufs=1, space="DRAM") as dram,
        ):
            # per-(row-block, col-tile) partial row sums from ACT accum_out
            acc = small.tile([128, rb * ct], f32)

            # constant bias AP for exp(x*scale - K)
            kbias = small.tile([128, 1], f32)
            nc.vector.memset(kbias[:], -K_SHIFT)

            # ---- bulk pass: exp(SCALE*x - K) summed along free axis ----
            for r in range(rb):
                for t in range(ct):
                    cos_tile = stream_pool.tile([128, f], f32, tag="stream")
                    nc.sync.dma_start(
                        out=cos_tile[:],
                        in_=cos_ext[r * 128 : (r + 1) * 128, t * f : (t + 1) * f],
                    )
                    i = r * ct + t
                    nc.scalar.activation(
                        cos_tile[:],
                        cos_tile[:],
                        AF.Exp,
                        bias=kbias[:],
                        scale=SCALE,
                        accum_out=acc[:, i : i + 1],
                    )

            # ---- local row sums: S_loc[p, r] = sum_t acc[p, r*ct + t] ----
            s_loc = small.tile([128, rb], f32)
            for r in range(rb):
                nc.vector.reduce_sum(
                    s_loc[:, r : r + 1],
                    acc[:, r * ct : (r + 1) * ct],
                    axis=mybir.AxisListType.X,
                )

            # ---- epilogue: margin correction for this core's rows ----
            xt_sb = small.tile([128, rb], f32)
            nc.sync.dma_start(out=xt_sb[:], in_=xt_ext[:])

            sq = small.tile([128, rb], f32)
            nc.vector.tensor_mul(sq[:], xt_sb[:], xt_sb[:])
            # sqrt(1 - xt^2)
            rt = small.tile([128, rb], f32)
            nc.scalar.activation(rt[:], sq[:], AF.Sqrt, bias=1.0, scale=-1.0)
            # lt = SCALE*cos_m*xt - SCALE*sin_m*sqrt(1-xt^2)
            t1 = small.tile([128, rb], f32)
            nc.vector.tensor_scalar_mul(t1[:], xt_sb[:], SCALE * cos_m)
            t2 = small.tile([128, rb], f32)
            nc.vector.tensor_scalar_mul(t2[:], rt[:], SCALE * sin_m)
            lt = small.tile([128, rb], f32)
            nc.vector.tensor_sub(lt[:], t1[:], t2[:])
            # e1 = exp(lt - K), e0 = exp(SCALE*xt - K)
            e1 = small.tile([128, rb], f32)
            nc.scalar.activation(e1[:], lt[:], AF.Exp, bias=kbias[:], scale=1.0)
            e0 = small.tile([128, rb], f32)
            nc.scalar.activation(e0[:], xt_sb[:], AF.Exp, bias=kbias[:], scale=SCALE)
            # S_true = S_loc - e0 + e1
            st = small.tile([128, rb], f32)
            nc.vector.tensor_sub(st[:], s_loc[:], e0[:])
            nc.vector.tensor_add(st[:], st[:], e1[:])
            # loss_b = log(S_true) + K - lt
            lg = small.tile([128, rb], f32)
            nc.scalar.activation(lg[:], st[:], AF.Ln)
            lossv = small.tile([128, rb], f32)
            nc.vector.tensor_sub(lossv[:], lg[:], lt[:])
            nc.vector.tensor_scalar_add(lossv[:], lossv[:], K_SHIFT)

            # ---- partition-sum via TensorE: [1, rb] = ones^T @ lossv ----
            ones = small.tile([128, 1], f32)
            nc.vector.memset(ones[:], 1.0)
            ps = psum.tile([1, rb], f32)
            nc.tensor.matmul(ps[:], ones[:], lossv[:])
            red = small.tile([1, 1], f32)
            nc.vector.reduce_sum(red[:], ps[:], axis=mybir.AxisListType.X)
            # partial mean contribution (divide by full B = n_cores*b_per)
            part = small.tile([1, 1], f32)
            nc.vector.tensor_scalar_mul(part[:], red[:], 1.0 / float(n_cores * b_per))

            # ---- AllReduce(add) the 8 partial means ----
            cc_in = dram.tile([1, 1], f32)
            cc_out = dram.tile([1, 1], f32)
            nc.sync.dma_start(out=cc_in[:], in_=part[:])
            nc.gpsimd.collective_compute(
                "AllReduce",
                mybir.AluOpType.add,
                replica_groups=[list(range(n_cores))],
                ins=[cc_in.opt()],
                outs=[cc_out.opt()],
            )
            nc.sync.dma_start(out=out_ext[:], in_=cc_out[:])

    nc.compile()
    return nc


def make_in_maps(cosine, label, b_per=B_PER, n_cores=N_CORES):
    """Host-side sharding: batch-split cosine (zero copy) + gather target
    cosines, laid out [128, rb] to match the device row layout."""
    cosine = np.ascontiguousarray(np.asarray(cosine, dtype=np.float32))
    label = np.asarray(label).astype(np.int64)
    b = cosine.shape[0]
    rb = b_per // 128
    xt = cosine[np.arange(b), label]  # [B] f32
    in_maps = []
    for i in range(n_cores):
        shard = cosine[i * b_per : (i + 1) * b_per]
        xtc = np.ascontiguousarray(xt[i * b_per : (i + 1) * b_per].reshape(rb, 128).T)
        in_maps.append({"cosine": shard, "xt": xtc})
    return in_maps


def kernel(cosine, label):
    from concourse.bass_utils import run_bass_kernel_spmd

    if "nc" not in _CACHE:
        _CACHE["nc"] = build_bass()
    nc = _CACHE["nc"]
    in_maps = make_in_maps(cosine, label)
    res = run_bass_kernel_spmd(nc, in_maps, core_ids=list(range(N_CORES)))
    out = np.asarray(res.results[0]["out"], dtype=np.float32).reshape(())
    return out
